# revision 7
# baseline (speedup 1.0000x reference)
"""Trainium2 Bass kernel for nn_HNet3_74801150427700 (topk_masking).

ref:  x = out.view(-1, 8); v = sort(x,1)[:, 3]  (4th smallest = lower median)
      y = softmax(x, 1) * (x > v)

Sharding: pure row-wise; rows split evenly across the 8 cores (data parallel,
no communication).  Inside a core: tiles of [128 partitions x 4096 fp32]
(each partition holds 512 contiguous groups of 8).

Engine split per tile:
  ScalarE (ACT): e = exp(x);  L = ln(s);  r = exp(-L) = 1/s
  GPSIMD:        group sums s via 3 pairwise-add levels (stride 8B reads)
  VectorE (DVE): pruned Batcher selection network for v (27 plane-ops),
                 q = e*r (bcast), d = x - v (bcast),
                 y = (d > 0) * q   (one scalar_tensor_tensor)
  DMA via HWDGE (nc.sync) both directions.
"""

import numpy as np

_NCORES = 8
_ROWS = 8388608
_K = 8
_P = 128
_C = 4096                    # fp32 per partition per tile
_F = _C // _K                # groups per partition per tile
_ELEMS_PER_CORE = _ROWS * _K // _NCORES      # 8388608
_NT = _ELEMS_PER_CORE // (_P * _C)           # 16 tiles

_nc_cache = {}


def _build(nt=_NT, c=_C):
    import concourse.bass as bass
    import concourse.bacc as bacc
    import concourse.mybir as mybir
    from contextlib import ExitStack
    from concourse.tile import TileContext

    f32 = mybir.dt.float32
    AF = mybir.ActivationFunctionType
    OP = mybir.AluOpType
    k = _K
    f = c // k

    nc = bacc.Bacc(None, target_bir_lowering=False)
    xd = nc.declare_dram_parameter("x", [nt, _P, c], f32, isOutput=False)
    yd = nc.declare_dram_parameter("y", [nt, _P, c], f32, isOutput=True)

    with TileContext(nc) as tc, ExitStack() as ctx:
        xp = ctx.enter_context(tc.tile_pool(name="xp", bufs=2))
        ep = ctx.enter_context(tc.tile_pool(name="ep", bufs=2))
        wp = ctx.enter_context(tc.tile_pool(name="wp", bufs=2))
        up = ctx.enter_context(tc.tile_pool(name="up", bufs=2))
        sp = ctx.enter_context(tc.tile_pool(name="sp", bufs=2))

        for t in range(nt):
            xt = xp.tile([_P, c], f32)
            nc.gpsimd.dma_start(out=xt[:], in_=xd[t])
            x3 = xt[:].rearrange("p (f k) -> p f k", k=k)

            et = ep.tile([_P, c], f32)
            nc.scalar.activation(et[:], xt[:], AF.Exp)
            e3 = et[:].rearrange("p (f k) -> p f k", k=k)

            # group sums: 3 pairwise levels on gpsimd
            s4 = sp.tile([_P, c // 2], f32, tag="s4")
            ee = et[:].rearrange("p (m two) -> p m two", two=2)
            nc.gpsimd.tensor_tensor(s4[:], ee[:, :, 0], ee[:, :, 1], op=OP.add)
            s2 = sp.tile([_P, c // 4], f32, tag="s2")
            s4v = s4[:].rearrange("p (m two) -> p m two", two=2)
            nc.gpsimd.tensor_tensor(s2[:], s4v[:, :, 0], s4v[:, :, 1], op=OP.add)
            s1 = sp.tile([_P, f], f32, tag="s1")
            s2v = s2[:].rearrange("p (m two) -> p m two", two=2)
            nc.gpsimd.tensor_tensor(s1[:], s2v[:, :, 0], s2v[:, :, 1], op=OP.add)

            # r = 1/s = exp(-ln(s)) on ACT (exp & ln share one table set)
            Lt = sp.tile([_P, f], f32, tag="L")
            nc.scalar.activation(Lt[:], s1[:], AF.Ln)
            rt = sp.tile([_P, f], f32, tag="r")
            nc.scalar.activation(rt[:], Lt[:], AF.Exp, scale=-1.0)

            # ---- selection network: v = 4th smallest of each group of 8 ----
            wt = wp.tile([_P, c], f32)
            ut = up.tile([_P, c], f32)
            w3 = wt[:].rearrange("p (f k) -> p f k", k=k)
            u3 = ut[:].rearrange("p (f k) -> p f k", k=k)
            # stage 1: CEs (0,1),(2,3),(4,5),(6,7)  [batched via pair view]
            xpair = xt[:].rearrange("p (m two) -> p m two", two=2)
            wpair = wt[:].rearrange("p (m two) -> p m two", two=2)
            nc.vector.tensor_tensor(wpair[:, :, 0], xpair[:, :, 0], xpair[:, :, 1], op=OP.min)
            nc.vector.tensor_tensor(wpair[:, :, 1], xpair[:, :, 0], xpair[:, :, 1], op=OP.max)
            # stage 2: (0,2),(1,3) then (4,6),(5,7)
            nc.vector.tensor_tensor(u3[:, :, 0:2], w3[:, :, 0:2], w3[:, :, 2:4], op=OP.min)
            nc.vector.tensor_tensor(u3[:, :, 2:4], w3[:, :, 0:2], w3[:, :, 2:4], op=OP.max)
            nc.vector.tensor_tensor(u3[:, :, 4:6], w3[:, :, 4:6], w3[:, :, 6:8], op=OP.min)
            nc.vector.tensor_tensor(u3[:, :, 6:8], w3[:, :, 4:6], w3[:, :, 6:8], op=OP.max)
            # stage 3: (1,2) and (5,6) -> sorted halves
            nc.vector.tensor_tensor(w3[:, :, 1], u3[:, :, 1], u3[:, :, 2], op=OP.min)
            nc.vector.tensor_tensor(w3[:, :, 2], u3[:, :, 1], u3[:, :, 2], op=OP.max)
            nc.vector.tensor_tensor(w3[:, :, 5], u3[:, :, 5], u3[:, :, 6], op=OP.min)
            nc.vector.tensor_tensor(w3[:, :, 6], u3[:, :, 5], u3[:, :, 6], op=OP.max)
            a0 = u3[:, :, 0]
            a1 = w3[:, :, 1]
            a2 = w3[:, :, 2]
            a3 = u3[:, :, 3]
            b0 = u3[:, :, 4]
            b1 = w3[:, :, 5]
            b2 = w3[:, :, 6]
            b3 = u3[:, :, 7]
            # pruned odd-even merge, output rank 3 (ascending) only:
            #   pos3' = min(min(a3,b3), max(a1,b1))
            #   pos4' = max(min(a2,b2), max(a0,b0))
            #   v     = min(pos3', pos4')
            t1 = sp.tile([_P, f], f32, tag="t1")
            nc.vector.tensor_tensor(t1[:], a0, b0, op=OP.max)
            t2 = sp.tile([_P, f], f32, tag="t2")
            nc.vector.tensor_tensor(t2[:], a1, b1, op=OP.max)
            t3 = sp.tile([_P, f], f32, tag="t3")
            nc.vector.tensor_tensor(t3[:], a2, b2, op=OP.min)
            t4 = sp.tile([_P, f], f32, tag="t4")
            nc.vector.tensor_tensor(t4[:], a3, b3, op=OP.min)
            p4 = sp.tile([_P, f], f32, tag="p4")
            nc.vector.tensor_tensor(p4[:], t3[:], t1[:], op=OP.max)
            p3 = sp.tile([_P, f], f32, tag="p3")
            nc.vector.tensor_tensor(p3[:], t4[:], t2[:], op=OP.min)
            vt = sp.tile([_P, f], f32, tag="v")
            nc.vector.tensor_tensor(vt[:], p3[:], p4[:], op=OP.min)

            # q = e * r  (r broadcast over the 8 group lanes); write into ut
            q3 = u3
            rb = rt[:].unsqueeze(2).broadcast_to([_P, f, k])
            nc.vector.tensor_tensor(q3, e3, rb, op=OP.mult)
            # d = x - v (broadcast); write into wt
            d3 = w3
            vb = vt[:].unsqueeze(2).broadcast_to([_P, f, k])
            nc.vector.tensor_tensor(d3, x3, vb, op=OP.subtract)
            # y = (d > 0) * q ; write into et (dead)
            nc.vector.scalar_tensor_tensor(
                et[:], wt[:], 0.0, ut[:], op0=OP.is_gt, op1=OP.mult
            )
            nc.gpsimd.dma_start(out=yd[t], in_=et[:])
    nc.finalize()
    return nc


def _get_nc(nt=_NT, c=_C):
    key = (nt, c)
    if key not in _nc_cache:
        _nc_cache[key] = _build(nt, c)
    return _nc_cache[key]


def _run(x_np, trace=False):
    """x_np: [ROWS, 8] fp32. Returns (y [ROWS,8] fp32, exec_time_ns|None)."""
    from concourse.bass_utils import run_bass_kernel_spmd

    nc = _get_nc()
    xs = np.ascontiguousarray(x_np, dtype=np.float32).reshape(
        _NCORES, _NT, _P, _C
    )
    in_maps = [{"x": xs[c]} for c in range(_NCORES)]
    out = run_bass_kernel_spmd(
        nc, in_maps, list(range(_NCORES)), trace=trace
    )
    y = np.stack([out.results[i]["y"] for i in range(_NCORES)])
    return y.reshape(_ROWS, _K), out.exec_time_ns


def _run_timed(x_np, iters=5):
    """Device-resident repeated execution; returns (y, mean_exec_seconds)."""
    import time

    import jax
    from jax.experimental.shard_map import shard_map
    from jax.sharding import Mesh, NamedSharding, PartitionSpec

    import concourse.mybir as mybir
    from concourse.bass2jax import (
        _bass_exec_p,
        install_neuronx_cc_hook,
        partition_id_tensor,
    )

    install_neuronx_cc_hook()
    nc = _get_nc()
    pname = nc.partition_id_tensor.name if nc.partition_id_tensor else None

    in_names, out_names, out_avals, zero_outs = [], [], [], []
    for alloc in nc.m.functions[0].allocations:
        if not isinstance(alloc, mybir.MemoryLocationSet):
            continue
        name = alloc.memorylocations[0].name
        if alloc.kind == "ExternalInput":
            if name != pname:
                in_names.append(name)
        elif alloc.kind == "ExternalOutput":
            out_names.append(name)
            shape = tuple(alloc.tensor_shape)
            dtype = mybir.dt.np(alloc.dtype)
            out_avals.append(jax.core.ShapedArray(shape, dtype))
            zero_outs.append(np.zeros(shape, dtype))
    n_params = len(in_names)
    all_in_names = in_names + out_names
    if pname is not None:
        all_in_names = all_in_names + [pname]

    def _body(*args):
        operands = list(args)
        if pname is not None:
            operands.append(partition_id_tensor())
        outs = _bass_exec_p.bind(
            *operands,
            out_avals=tuple(out_avals),
            in_names=tuple(all_in_names),
            out_names=tuple(out_names),
            lowering_input_output_aliases=(),
            sim_require_finite=True,
            sim_require_nnan=True,
            nc=nc,
        )
        return tuple(outs)

    xs = np.ascontiguousarray(x_np, dtype=np.float32).reshape(
        _NCORES, _NT, _P, _C
    )
    devices = jax.devices()[:_NCORES]
    mesh = Mesh(np.asarray(devices), ("core",))
    spec = PartitionSpec("core")
    n_outs = len(out_names)
    sharded = jax.jit(
        shard_map(
            _body,
            mesh=mesh,
            in_specs=(spec,) * (n_params + n_outs),
            out_specs=(spec,) * n_outs,
            check_rep=False,
        ),
        keep_unused=True,
    )
    sh = NamedSharding(mesh, spec)
    xin = jax.device_put(xs.reshape(_NCORES * _NT, _P, _C), sh)
    zin = [
        jax.device_put(
            np.zeros((_NCORES * z.shape[0], *z.shape[1:]), z.dtype), sh
        )
        for z in zero_outs
    ]
    outs = sharded(xin, *zin)
    jax.block_until_ready(outs)
    t0 = time.perf_counter()
    for _ in range(iters):
        outs = sharded(xin, *zin)
        jax.block_until_ready(outs)
    dt = (time.perf_counter() - t0) / iters
    y = np.asarray(outs[0]).reshape(_ROWS, _K)
    return y, dt


def kernel(out, num_per_group):
    x = np.asarray(out, dtype=np.float32)
    assert x.shape == (_ROWS, _K), x.shape
    assert int(num_per_group) == _K
    y, _ = _run(x)
    return y
